# revision 18
# baseline (speedup 1.0000x reference)
"""DenseNet block (6 layers, growth 32) on 8 TRN2 NeuronCores.

Data-parallel over batch: 32 images -> 4 per core. Per core, per image:
  features live in SBUF as f32, channels on partitions, spatial flattened
  in a zero-padded 58x58 layout; channel count grows 256 -> 448.
  Per layer:
    h = relu(feat * scale + shift)  (BN folded host-side) -> fp16, computed
      on ScalarE (activation) with a slice on VectorE (tensor_scalar 2x_2P),
      into parity-alternating zero-bordered h buffers. The freshest 32
      channels are instead produced per-tile right after their conv.
    conv 3x3 = 9 shifted matmuls per 128-channel K-chunk over row-aligned
      spatial tiles (8 rows x 58 = 464 cols = one PSUM bank), fp16 operands,
      f32 PSUM accumulation. M=32 output channels -> 4-way col-group packing
      (tile_position=(0,32g)) over (offset, chunk) instances; K always padded
      to 128 (zero weights) because sub-128-row PE tiles serialize a round.
      A selection-matrix matmul (M=32) reduces the 4 partials onto the
      partition range where the new channels live; VectorE evacuates PSUM.
  Tails (cast/reduce/copy/fused-relu) trail the matmul rounds by 3 tiles so
  the PE never waits on PSUM evacuation.
  Output: the 192 new channels DMA out padded; the host slices the interior
  and prepends x unchanged.
"""

import sys
import types

import numpy as np

import concourse.bass as bass
import concourse.tile as tile
from concourse import bacc, mybir
from concourse.bass_utils import run_bass_kernel_spmd

# ---------------------------------------------------------------- constants
NUM_LAYERS = 6
C_IN = 256
GROWTH = 32
EPS = 1e-5
N_CORES = 8
IMGS_PER_CORE = 4
H = W = 56
HP = H + 2  # 58
SP = HP * HP  # 3364 padded spatial
GUARD = 64
SPG = GUARD + SP + GUARD
ROWS_PER_TILE = 8
TILE_N = ROWS_PER_TILE * HP  # 464, row-aligned: tile t covers grid rows 1+8t..9+8t
N_TILES = H // ROWS_PER_TILE  # 7 (covers all 56 interior rows)
FP16 = mybir.dt.float16
F32 = mybir.dt.float32

LAYER_C = [C_IN + i * GROWTH for i in range(NUM_LAYERS)]  # 256..416
LAYER_CHUNKS = [
    [128] * (c // 128) + ([c % 128] if c % 128 else []) for c in LAYER_C
]
P0 = [(C_IN + i * GROWTH) % 128 for i in range(NUM_LAYERS)]  # 0,32,64,96,0,32
CI = [(C_IN + i * GROWTH) // 128 for i in range(NUM_LAYERS)]  # 2,2,2,2,3,3
OFFS = [HP * (dy - 1) + (dx - 1) for dy in range(3) for dx in range(3)]
# per-layer (offset, chunk) instance lists and global weight-column indexes
LAYER_INSTS = []
_idx = 0
LAYER_IBASE = []
for _i in range(NUM_LAYERS):
    LAYER_IBASE.append(_idx)
    insts = [(o, c) for o in range(9) for c in range(len(LAYER_CHUNKS[_i]))]
    LAYER_INSTS.append(insts)
    _idx += len(insts)
N_INST = _idx  # 162

_COMPILED = None


def _install_ntff_hook():
    """Register the axon NTFF profile hook if the image's antenv lacks it."""
    try:
        import antenv.axon_hooks  # noqa: F401

        return
    except ImportError:
        pass
    try:
        import antenv
        from trn_agent_boot.trn_boot import _ntff_profile_via_ctypes

        hooks = types.ModuleType("antenv.axon_hooks")
        store = {}
        hooks.set_axon_ntff_profile_hook = lambda h: store.__setitem__("h", h)
        hooks.get_axon_ntff_profile_hook = lambda: store.get("h")
        antenv.axon_hooks = hooks
        sys.modules["antenv.axon_hooks"] = hooks
        hooks.set_axon_ntff_profile_hook(
            _ntff_profile_via_ctypes("/opt/axon/libaxon_pjrt.so")
        )
    except Exception:
        pass


def _interior(ap):
    """[P, SP]-flat AP -> [P, 56, 56] interior view of the padded 58x58 grid."""
    return ap.rearrange("p (r q) -> p r q", q=HP)[:, 1 : 1 + H, 1 : 1 + W]


def _build():
    nc = bacc.Bacc(None, target_bir_lowering=False, debug=False)

    x_d = nc.declare_dram_parameter(
        "x", [IMGS_PER_CORE, 2, 128, SP], F32, isOutput=False
    )
    w_d = nc.declare_dram_parameter("w", [128, N_INST, 32], FP16, isOutput=False)
    sel_d = nc.declare_dram_parameter("sel", [128, 4, 128], FP16, isOutput=False)
    bn_d = nc.declare_dram_parameter(
        "bn", [128, NUM_LAYERS, 4, 2], F32, isOutput=False
    )
    out_d = nc.declare_dram_parameter(
        "out", [IMGS_PER_CORE, 192, SP], F32, isOutput=True
    )

    with tile.TileContext(nc) as tc:
        with (
            tc.tile_pool(name="persist", bufs=1) as persist,
            tc.tile_pool(name="stage", bufs=8) as stage_pool,
            tc.tile_pool(name="acc", bufs=4, space="PSUM") as acc_pool,
            tc.tile_pool(name="red", bufs=4, space="PSUM") as red_pool,
        ):
            w_sb = persist.tile([128, N_INST, 32], FP16)
            sel_sb = persist.tile([128, 4, 128], FP16)
            bn_sb = persist.tile([128, NUM_LAYERS, 4, 2], F32)
            # x chunks double-buffered by image parity; grown chunks 2,3
            xbuf = persist.tile([128, 2, 2, SP], F32)
            grow = persist.tile([128, 2, SP], F32)
            hbuf = [persist.tile([128, 4, SPG], FP16, name=f"h{p}") for p in (0, 1, 2)]

            def feat_chunk(img, c):
                return xbuf[:, img % 2, c, :] if c < 2 else grow[:, c - 2, :]

            nc.sync.dma_start(out=w_sb[:], in_=w_d[:])
            nc.sync.dma_start(out=sel_sb[:], in_=sel_d[:])
            nc.sync.dma_start(out=bn_sb[:], in_=bn_d[:])
            # full zero-init: K-padded matmuls read every row, and
            # uninitialized SBUF can hold NaN patterns (NaN * 0 = NaN)
            for hb in hbuf:
                nc.gpsimd.memset(hb[:], 0.0)
            # grow border rows are DMA'd out (host slices them off) but
            # never computed; zero once so reads are initialized
            nc.vector.memset(grow[:, :, 0:HP], 0.0)
            nc.vector.memset(grow[:, :, HP * (HP - 1) :], 0.0)

            def tile_range(t):
                """Flat offset (into SP) and length of row-aligned tile t."""
                return HP * (1 + ROWS_PER_TILE * t), TILE_N

            def rowwise(ap_flat):
                """[P, TILE_N] flat window -> [P, 8, 56] interior-columns view."""
                return ap_flat.rearrange("p (r q) -> p r q", q=HP)[:, :, 1 : 1 + W]

            for img in range(IMGS_PER_CORE):
                # load x (interior of padded layout), double-buffered by parity
                for c in range(2):
                    nc.sync.dma_start(
                        out=feat_chunk(img, c), in_=x_d[img, c]
                    )

                def emit_tail(i, t):
                    """Post-round ops for (layer i, tile t): cast partials,
                    selection-reduce, copy raw to feat, fused BN+ReLU of the
                    fresh 32 channels into the NEXT layer's h buffer."""
                    p0, ci = P0[i], CI[i]
                    acc, stg, red = tile_state[(i, t)]
                    o0, nt = tile_range(t)
                    nc.vector.tensor_copy(out=stg[:, :nt], in_=acc[:, :nt])
                    nc.tensor.matmul(
                        out=red[p0 : p0 + 32, :nt],
                        lhsT=sel_sb[:, p0 // 32, p0 : p0 + 32],
                        rhs=stg[:, :nt],
                        start=True,
                        stop=True,
                        tile_position=(0, p0),
                    )
                    nc.vector.tensor_copy(
                        out=grow[p0 : p0 + 32, ci - 2, o0 : o0 + nt],
                        in_=red[p0 : p0 + 32, :nt],
                    )
                    if i + 1 < NUM_LAYERS:
                        hn = hbuf[(i + 1) % 3]
                        nc.scalar.activation(
                            out=rowwise(
                                hn[p0 : p0 + 32, ci, GUARD + o0 : GUARD + o0 + nt]
                            ),
                            in_=rowwise(grow[p0 : p0 + 32, ci - 2, o0 : o0 + nt]),
                            func=mybir.ActivationFunctionType.Relu,
                            scale=bn_sb[p0 : p0 + 32, i + 1, ci, 0:1],
                            bias=bn_sb[p0 : p0 + 32, i + 1, ci, 1:2],
                        )

                tile_state = {}
                pending = []  # (layer, tile) whose tails are not yet emitted
                for i in range(NUM_LAYERS):
                    chunks = LAYER_CHUNKS[i]
                    h = hbuf[i % 3]
                    # full-chunk BN+ReLU for all but the freshest 32 channels
                    # (those were produced fused from PSUM by the prior layer);
                    # split into two row-halves for finer-grained pipelining
                    for c, kc in enumerate(chunks):
                        if i >= 1 and c == CI[i - 1]:
                            kc = P0[i - 1]  # exclude prior layer's fresh rows
                        if kc == 0:
                            continue
                        fc = feat_chunk(img, c)
                        for hi, (r0, r1) in enumerate(((0, H // 2), (H // 2, H))):
                            src_v = fc.rearrange("p (r q) -> p r q", q=HP)[
                                :kc, 1 + r0 : 1 + r1, 1 : 1 + W
                            ]
                            dst_v = h[:kc, c, GUARD : GUARD + SP].rearrange(
                                "p (r q) -> p r q", q=HP
                            )[:, 1 + r0 : 1 + r1, 1 : 1 + W]
                            if c == 1 and hi == 1:
                                # DVE path: fp32 tensor_scalar runs 2x_2P,
                                # fp16 in-place max runs 4x
                                nc.vector.tensor_scalar(
                                    out=dst_v,
                                    in0=src_v,
                                    scalar1=bn_sb[:kc, i, c, 0:1],
                                    scalar2=bn_sb[:kc, i, c, 1:2],
                                    op0=mybir.AluOpType.mult,
                                    op1=mybir.AluOpType.add,
                                )
                                nc.vector.tensor_scalar_max(
                                    out=dst_v, in0=dst_v, scalar1=0.0
                                )
                            else:
                                nc.scalar.activation(
                                    out=dst_v,
                                    in_=src_v,
                                    func=mybir.ActivationFunctionType.Relu,
                                    scale=bn_sb[:kc, i, c, 0:1],
                                    bias=bn_sb[:kc, i, c, 1:2],
                                )

                    insts = LAYER_INSTS[i]
                    ibase = LAYER_IBASE[i]
                    ngrp = [len(insts[g::4]) for g in range(4)]
                    for t in range(N_TILES):
                        o0, nt = tile_range(t)
                        hb0 = GUARD + o0
                        acc = acc_pool.tile([128, TILE_N], F32)
                        stg = stage_pool.tile([128, TILE_N], FP16)
                        red = red_pool.tile([128, TILE_N], F32)
                        tile_state[(i, t)] = (acc, stg, red)
                        # round-major emission: groups run concurrently.
                        # All K padded to 128 (zero weights / zero h rows):
                        # sub-128-row tiles serialize the whole round on HW.
                        for r in range(max(ngrp)):
                            for g in range(4):
                                j = r * 4 + g
                                if j >= len(insts):
                                    continue
                                o, c = insts[j]
                                nc.tensor.matmul(
                                    out=acc[32 * g : 32 * g + 32, :nt],
                                    lhsT=w_sb[:, ibase + j, :],
                                    rhs=h[:, c, hb0 + OFFS[o] : hb0 + OFFS[o] + nt],
                                    start=(r == 0),
                                    stop=(r == ngrp[g] - 1),
                                    tile_position=(0, 32 * g),
                                    skip_group_check=True,
                                )
                        # tails trail the rounds by two tiles so their
                        # PSUM->SBUF casts never stall the PE
                        pending.append((i, t))
                        if len(pending) > 3:
                            emit_tail(*pending.pop(0))
                for p in pending:
                    emit_tail(*p)

                # write out the 192 new channels (padded; host slices)
                nc.sync.dma_start(out=out_d[img, 0:128], in_=grow[:, 0, :])
                nc.sync.dma_start(out=out_d[img, 128:192], in_=grow[:64, 1, :])

    nc.compile()
    return nc


def _get_compiled():
    global _COMPILED
    if _COMPILED is None:
        _COMPILED = _build()
    return _COMPILED


def _prep_shared(bn_weights, bn_biases, bn_means, bn_vars, conv_kernels):
    """Weights / selection / BN tensors (identical on every core)."""
    w_all = np.zeros((128, N_INST, 32), dtype=np.float16)
    for i in range(NUM_LAYERS):
        wk = np.asarray(conv_kernels[i], dtype=np.float32)  # [32, C, 3, 3]
        for j, (o, c) in enumerate(LAYER_INSTS[i]):
            dy, dx = o // 3, o % 3
            kc = LAYER_CHUNKS[i][c]
            blk = wk[:, 128 * c : 128 * c + kc, dy, dx]  # [32, kc]
            w_all[:kc, LAYER_IBASE[i] + j, :] = blk.T.astype(np.float16)

    sel = np.zeros((128, 4, 128), dtype=np.float16)
    for v in range(4):
        for g in range(4):
            for m in range(32):
                sel[32 * g + m, v, 32 * v + m] = 1.0

    bn = np.zeros((128, NUM_LAYERS, 4, 2), dtype=np.float32)
    for i in range(NUM_LAYERS):
        s = np.asarray(bn_weights[i], np.float32) / np.sqrt(
            np.asarray(bn_vars[i], np.float32) + EPS
        )
        t = np.asarray(bn_biases[i], np.float32) - np.asarray(
            bn_means[i], np.float32
        ) * s
        for c, kc in enumerate(LAYER_CHUNKS[i]):
            bn[:kc, i, c, 0] = s[128 * c : 128 * c + kc]
            bn[:kc, i, c, 1] = t[128 * c : 128 * c + kc]
    return w_all, sel, bn


def _run(inputs, trace=False):
    _install_ntff_hook()
    nc = _get_compiled()
    x = np.asarray(inputs["x"], dtype=np.float32)
    n = x.shape[0]
    w_all, sel, bn = _prep_shared(
        inputs["bn_weights"],
        inputs["bn_biases"],
        inputs["bn_means"],
        inputs["bn_vars"],
        inputs["conv_kernels"],
    )
    xr = x.reshape(n, 2, 128, H, W)
    xpad = np.zeros((n, 2, 128, HP, HP), dtype=np.float32)
    xpad[:, :, :, 1 : 1 + H, 1 : 1 + W] = xr
    xpad = xpad.reshape(n, 2, 128, SP)
    in_maps = []
    for core in range(N_CORES):
        sl = xpad[core * IMGS_PER_CORE : (core + 1) * IMGS_PER_CORE]
        in_maps.append(
            {"x": np.ascontiguousarray(sl), "w": w_all, "sel": sel, "bn": bn}
        )
    res = run_bass_kernel_spmd(nc, in_maps, list(range(N_CORES)), trace=trace)
    out = np.empty((n, C_IN + NUM_LAYERS * GROWTH, H, W), dtype=np.float32)
    out[:, :C_IN] = x.reshape(n, C_IN, H, W)
    for core in range(N_CORES):
        new = res.results[core]["out"].reshape(IMGS_PER_CORE, 192, HP, HP)
        out[core * IMGS_PER_CORE : (core + 1) * IMGS_PER_CORE, C_IN:] = new[
            :, :, 1 : 1 + H, 1 : 1 + W
        ]
    return out, res


def kernel(**inputs) -> np.ndarray:
    out, _ = _run(inputs, trace=False)
    return out


# revision 19
# speedup vs baseline: 1.0320x; 1.0320x over previous
"""DenseNet block (6 layers, growth 32) on 8 TRN2 NeuronCores.

Data-parallel over batch: 32 images -> 4 per core. Per core, per image:
  features live in SBUF as f32, channels on partitions, spatial flattened
  in a zero-padded 58x58 layout; channel count grows 256 -> 448.
  Per layer:
    h = relu(feat * scale + shift)  (BN folded host-side) -> fp16, computed
      on ScalarE (activation) with a slice on VectorE (tensor_scalar 2x_2P),
      into parity-alternating zero-bordered h buffers. The freshest 32
      channels are instead produced per-tile right after their conv.
    conv 3x3 = 9 shifted matmuls per 128-channel K-chunk over row-aligned
      spatial tiles (8 rows x 58 = 464 cols = one PSUM bank), fp16 operands,
      f32 PSUM accumulation. M=32 output channels -> 4-way col-group packing
      (tile_position=(0,32g)) over (offset, chunk) instances; K always padded
      to 128 (zero weights) because sub-128-row PE tiles serialize a round.
      A selection-matrix matmul (M=32) reduces the 4 partials onto the
      partition range where the new channels live; VectorE evacuates PSUM.
  Tails (cast/reduce/copy/fused-relu) trail the matmul rounds by 3 tiles so
  the PE never waits on PSUM evacuation.
  Output: the 192 new channels DMA out padded; the host slices the interior
  and prepends x unchanged.
"""

import sys
import types

import numpy as np

import concourse.bass as bass
import concourse.tile as tile
from concourse import bacc, mybir
from concourse.bass_utils import run_bass_kernel_spmd

# ---------------------------------------------------------------- constants
NUM_LAYERS = 6
C_IN = 256
GROWTH = 32
EPS = 1e-5
N_CORES = 8
IMGS_PER_CORE = 4
H = W = 56
HP = H + 2  # 58
SP = HP * HP  # 3364 padded spatial
GUARD = 64
SPG = GUARD + SP + GUARD
ROWS_PER_TILE = 8
TILE_N = ROWS_PER_TILE * HP  # 464, row-aligned: tile t covers grid rows 1+8t..9+8t
N_TILES = H // ROWS_PER_TILE  # 7 (covers all 56 interior rows)
FP16 = mybir.dt.float16
F32 = mybir.dt.float32

LAYER_C = [C_IN + i * GROWTH for i in range(NUM_LAYERS)]  # 256..416
LAYER_CHUNKS = [
    [128] * (c // 128) + ([c % 128] if c % 128 else []) for c in LAYER_C
]
P0 = [(C_IN + i * GROWTH) % 128 for i in range(NUM_LAYERS)]  # 0,32,64,96,0,32
CI = [(C_IN + i * GROWTH) // 128 for i in range(NUM_LAYERS)]  # 2,2,2,2,3,3
OFFS = [HP * (dy - 1) + (dx - 1) for dy in range(3) for dx in range(3)]
# per-layer (offset, chunk) instance lists and global weight-column indexes
LAYER_INSTS = []
_idx = 0
LAYER_IBASE = []
for _i in range(NUM_LAYERS):
    LAYER_IBASE.append(_idx)
    insts = [(o, c) for o in range(9) for c in range(len(LAYER_CHUNKS[_i]))]
    LAYER_INSTS.append(insts)
    _idx += len(insts)
N_INST = _idx  # 162

_COMPILED = None


def _install_ntff_hook():
    """Register the axon NTFF profile hook if the image's antenv lacks it."""
    try:
        import antenv.axon_hooks  # noqa: F401

        return
    except ImportError:
        pass
    try:
        import antenv
        from trn_agent_boot.trn_boot import _ntff_profile_via_ctypes

        hooks = types.ModuleType("antenv.axon_hooks")
        store = {}
        hooks.set_axon_ntff_profile_hook = lambda h: store.__setitem__("h", h)
        hooks.get_axon_ntff_profile_hook = lambda: store.get("h")
        antenv.axon_hooks = hooks
        sys.modules["antenv.axon_hooks"] = hooks
        hooks.set_axon_ntff_profile_hook(
            _ntff_profile_via_ctypes("/opt/axon/libaxon_pjrt.so")
        )
    except Exception:
        pass


def _interior(ap):
    """[P, SP]-flat AP -> [P, 56, 56] interior view of the padded 58x58 grid."""
    return ap.rearrange("p (r q) -> p r q", q=HP)[:, 1 : 1 + H, 1 : 1 + W]


def _build():
    nc = bacc.Bacc(None, target_bir_lowering=False, debug=False)

    x_d = nc.declare_dram_parameter(
        "x", [IMGS_PER_CORE, 2, 128, SP], F32, isOutput=False
    )
    w_d = nc.declare_dram_parameter("w", [128, N_INST, 32], FP16, isOutput=False)
    sel_d = nc.declare_dram_parameter("sel", [128, 4, 128], FP16, isOutput=False)
    bn_d = nc.declare_dram_parameter(
        "bn", [128, NUM_LAYERS, 4, 2], F32, isOutput=False
    )
    out_d = nc.declare_dram_parameter(
        "out", [IMGS_PER_CORE, 192, SP], F32, isOutput=True
    )

    with tile.TileContext(nc) as tc:
        with (
            tc.tile_pool(name="persist", bufs=1) as persist,
            tc.tile_pool(name="stage", bufs=8) as stage_pool,
            tc.tile_pool(name="acc", bufs=4, space="PSUM") as acc_pool,
            tc.tile_pool(name="red", bufs=4, space="PSUM") as red_pool,
        ):
            w_sb = persist.tile([128, N_INST, 32], FP16)
            sel_sb = persist.tile([128, 4, 128], FP16)
            bn_sb = persist.tile([128, NUM_LAYERS, 4, 2], F32)
            # x chunks double-buffered by image parity; grown chunks 2,3
            xbuf = persist.tile([128, 2, 2, SP], F32)
            grow = persist.tile([128, 2, SP], F32)
            hbuf = [persist.tile([128, 4, SPG], FP16, name=f"h{p}") for p in (0, 1)]

            def feat_chunk(img, c):
                return xbuf[:, img % 2, c, :] if c < 2 else grow[:, c - 2, :]

            nc.sync.dma_start(out=w_sb[:], in_=w_d[:])
            nc.sync.dma_start(out=sel_sb[:], in_=sel_d[:])
            nc.sync.dma_start(out=bn_sb[:], in_=bn_d[:])
            # full zero-init: K-padded matmuls read every row, and
            # uninitialized SBUF can hold NaN patterns (NaN * 0 = NaN)
            for hb in hbuf:
                nc.gpsimd.memset(hb[:], 0.0)
            # grow border rows are DMA'd out (host slices them off) but
            # never computed; zero once so reads are initialized
            nc.vector.memset(grow[:, :, 0:HP], 0.0)
            nc.vector.memset(grow[:, :, HP * (HP - 1) :], 0.0)

            def tile_range(t):
                """Flat offset (into SP) and length of row-aligned tile t."""
                return HP * (1 + ROWS_PER_TILE * t), TILE_N

            def rowwise(ap_flat):
                """[P, TILE_N] flat window -> [P, 8, 56] interior-columns view."""
                return ap_flat.rearrange("p (r q) -> p r q", q=HP)[:, :, 1 : 1 + W]

            for img in range(IMGS_PER_CORE):
                # load x (interior of padded layout), double-buffered by parity
                for c in range(2):
                    nc.sync.dma_start(
                        out=feat_chunk(img, c), in_=x_d[img, c]
                    )

                def emit_tail(i, t):
                    """Post-round ops for (layer i, tile t): cast partials,
                    selection-reduce, copy raw to feat, fused BN+ReLU of the
                    fresh 32 channels into the NEXT layer's h buffer."""
                    p0, ci = P0[i], CI[i]
                    acc, stg, red = tile_state[(i, t)]
                    o0, nt = tile_range(t)
                    nc.vector.tensor_copy(out=stg[:, :nt], in_=acc[:, :nt])
                    nc.tensor.matmul(
                        out=red[p0 : p0 + 32, :nt],
                        lhsT=sel_sb[:, p0 // 32, p0 : p0 + 32],
                        rhs=stg[:, :nt],
                        start=True,
                        stop=True,
                        tile_position=(0, p0),
                    )
                    nc.vector.tensor_copy(
                        out=grow[p0 : p0 + 32, ci - 2, o0 : o0 + nt],
                        in_=red[p0 : p0 + 32, :nt],
                    )
                    if i + 1 < NUM_LAYERS:
                        hn = hbuf[(i + 1) % 2]
                        nc.scalar.activation(
                            out=rowwise(
                                hn[p0 : p0 + 32, ci, GUARD + o0 : GUARD + o0 + nt]
                            ),
                            in_=rowwise(grow[p0 : p0 + 32, ci - 2, o0 : o0 + nt]),
                            func=mybir.ActivationFunctionType.Relu,
                            scale=bn_sb[p0 : p0 + 32, i + 1, ci, 0:1],
                            bias=bn_sb[p0 : p0 + 32, i + 1, ci, 1:2],
                        )

                tile_state = {}
                pending = []  # (layer, tile) whose tails are not yet emitted
                for i in range(NUM_LAYERS):
                    chunks = LAYER_CHUNKS[i]
                    h = hbuf[i % 2]
                    # full-chunk BN+ReLU for all but the freshest 32 channels
                    # (those were produced fused from PSUM by the prior layer);
                    # split into two row-halves for finer-grained pipelining
                    for c, kc in enumerate(chunks):
                        if i >= 1 and c == CI[i - 1]:
                            kc = P0[i - 1]  # exclude prior layer's fresh rows
                        if kc == 0:
                            continue
                        fc = feat_chunk(img, c)
                        for hi, (r0, r1) in enumerate(((0, H // 2), (H // 2, H))):
                            src_v = fc.rearrange("p (r q) -> p r q", q=HP)[
                                :kc, 1 + r0 : 1 + r1, 1 : 1 + W
                            ]
                            dst_v = h[:kc, c, GUARD : GUARD + SP].rearrange(
                                "p (r q) -> p r q", q=HP
                            )[:, 1 + r0 : 1 + r1, 1 : 1 + W]
                            if c == 1 and hi == 1:
                                # DVE path: fp32 tensor_scalar runs 2x_2P,
                                # fp16 in-place max runs 4x
                                nc.vector.tensor_scalar(
                                    out=dst_v,
                                    in0=src_v,
                                    scalar1=bn_sb[:kc, i, c, 0:1],
                                    scalar2=bn_sb[:kc, i, c, 1:2],
                                    op0=mybir.AluOpType.mult,
                                    op1=mybir.AluOpType.add,
                                )
                                nc.vector.tensor_scalar_max(
                                    out=dst_v, in0=dst_v, scalar1=0.0
                                )
                            else:
                                nc.scalar.activation(
                                    out=dst_v,
                                    in_=src_v,
                                    func=mybir.ActivationFunctionType.Relu,
                                    scale=bn_sb[:kc, i, c, 0:1],
                                    bias=bn_sb[:kc, i, c, 1:2],
                                )

                    insts = LAYER_INSTS[i]
                    ibase = LAYER_IBASE[i]
                    ngrp = [len(insts[g::4]) for g in range(4)]
                    for t in range(N_TILES):
                        o0, nt = tile_range(t)
                        hb0 = GUARD + o0
                        acc = acc_pool.tile([128, TILE_N], F32)
                        stg = stage_pool.tile([128, TILE_N], FP16)
                        red = red_pool.tile([128, TILE_N], F32)
                        tile_state[(i, t)] = (acc, stg, red)
                        # round-major emission: groups run concurrently.
                        # All K padded to 128 (zero weights / zero h rows):
                        # sub-128-row tiles serialize the whole round on HW.
                        for r in range(max(ngrp)):
                            for g in range(4):
                                j = r * 4 + g
                                if j >= len(insts):
                                    continue
                                o, c = insts[j]
                                nc.tensor.matmul(
                                    out=acc[32 * g : 32 * g + 32, :nt],
                                    lhsT=w_sb[:, ibase + j, :],
                                    rhs=h[:, c, hb0 + OFFS[o] : hb0 + OFFS[o] + nt],
                                    start=(r == 0),
                                    stop=(r == ngrp[g] - 1),
                                    tile_position=(0, 32 * g),
                                    skip_group_check=True,
                                )
                        # tails trail the rounds by two tiles so their
                        # PSUM->SBUF casts never stall the PE
                        pending.append((i, t))
                        if len(pending) > 3:
                            emit_tail(*pending.pop(0))
                for p in pending:
                    emit_tail(*p)

                # write out the 192 new channels (padded; host slices)
                nc.sync.dma_start(out=out_d[img, 0:128], in_=grow[:, 0, :])
                nc.sync.dma_start(out=out_d[img, 128:192], in_=grow[:64, 1, :])

    nc.compile()
    return nc


def _get_compiled():
    global _COMPILED
    if _COMPILED is None:
        _COMPILED = _build()
    return _COMPILED


def _prep_shared(bn_weights, bn_biases, bn_means, bn_vars, conv_kernels):
    """Weights / selection / BN tensors (identical on every core)."""
    w_all = np.zeros((128, N_INST, 32), dtype=np.float16)
    for i in range(NUM_LAYERS):
        wk = np.asarray(conv_kernels[i], dtype=np.float32)  # [32, C, 3, 3]
        for j, (o, c) in enumerate(LAYER_INSTS[i]):
            dy, dx = o // 3, o % 3
            kc = LAYER_CHUNKS[i][c]
            blk = wk[:, 128 * c : 128 * c + kc, dy, dx]  # [32, kc]
            w_all[:kc, LAYER_IBASE[i] + j, :] = blk.T.astype(np.float16)

    sel = np.zeros((128, 4, 128), dtype=np.float16)
    for v in range(4):
        for g in range(4):
            for m in range(32):
                sel[32 * g + m, v, 32 * v + m] = 1.0

    bn = np.zeros((128, NUM_LAYERS, 4, 2), dtype=np.float32)
    for i in range(NUM_LAYERS):
        s = np.asarray(bn_weights[i], np.float32) / np.sqrt(
            np.asarray(bn_vars[i], np.float32) + EPS
        )
        t = np.asarray(bn_biases[i], np.float32) - np.asarray(
            bn_means[i], np.float32
        ) * s
        for c, kc in enumerate(LAYER_CHUNKS[i]):
            bn[:kc, i, c, 0] = s[128 * c : 128 * c + kc]
            bn[:kc, i, c, 1] = t[128 * c : 128 * c + kc]
    return w_all, sel, bn


def _run(inputs, trace=False):
    _install_ntff_hook()
    nc = _get_compiled()
    x = np.asarray(inputs["x"], dtype=np.float32)
    n = x.shape[0]
    w_all, sel, bn = _prep_shared(
        inputs["bn_weights"],
        inputs["bn_biases"],
        inputs["bn_means"],
        inputs["bn_vars"],
        inputs["conv_kernels"],
    )
    xr = x.reshape(n, 2, 128, H, W)
    xpad = np.zeros((n, 2, 128, HP, HP), dtype=np.float32)
    xpad[:, :, :, 1 : 1 + H, 1 : 1 + W] = xr
    xpad = xpad.reshape(n, 2, 128, SP)
    in_maps = []
    for core in range(N_CORES):
        sl = xpad[core * IMGS_PER_CORE : (core + 1) * IMGS_PER_CORE]
        in_maps.append(
            {"x": np.ascontiguousarray(sl), "w": w_all, "sel": sel, "bn": bn}
        )
    res = run_bass_kernel_spmd(nc, in_maps, list(range(N_CORES)), trace=trace)
    out = np.empty((n, C_IN + NUM_LAYERS * GROWTH, H, W), dtype=np.float32)
    out[:, :C_IN] = x.reshape(n, C_IN, H, W)
    for core in range(N_CORES):
        new = res.results[core]["out"].reshape(IMGS_PER_CORE, 192, HP, HP)
        out[core * IMGS_PER_CORE : (core + 1) * IMGS_PER_CORE, C_IN:] = new[
            :, :, 1 : 1 + H, 1 : 1 + W
        ]
    return out, res


def kernel(**inputs) -> np.ndarray:
    out, _ = _run(inputs, trace=False)
    return out


# revision 20
# speedup vs baseline: 1.0417x; 1.0094x over previous
"""DenseNet block (6 layers, growth 32) on 8 TRN2 NeuronCores.

Data-parallel over batch: 32 images -> 4 per core. Per core, per image:
  features live in SBUF as f32, channels on partitions, spatial flattened
  in a zero-padded 58x58 layout; channel count grows 256 -> 448.
  Per layer:
    h = relu(feat * scale + shift)  (BN folded host-side) -> fp16, computed
      on ScalarE (activation) with a slice on VectorE (tensor_scalar 2x_2P),
      into parity-alternating zero-bordered h buffers. The freshest 32
      channels are instead produced per-tile right after their conv.
    conv 3x3 = 9 shifted matmuls per 128-channel K-chunk over row-aligned
      spatial tiles (8 rows x 58 = 464 cols = one PSUM bank), fp16 operands,
      f32 PSUM accumulation. M=32 output channels -> 4-way col-group packing
      (tile_position=(0,32g)) over (offset, chunk) instances; K always padded
      to 128 (zero weights) because sub-128-row PE tiles serialize a round.
      A selection-matrix matmul (M=32) reduces the 4 partials onto the
      partition range where the new channels live; VectorE evacuates PSUM.
  Tails (cast/reduce/copy/fused-relu) trail the matmul rounds by 3 tiles so
  the PE never waits on PSUM evacuation.
  Output: the 192 new channels DMA out padded; the host slices the interior
  and prepends x unchanged.
"""

import sys
import types

import numpy as np

import concourse.bass as bass
import concourse.tile as tile
from concourse import bacc, mybir
from concourse.bass_utils import run_bass_kernel_spmd

# ---------------------------------------------------------------- constants
NUM_LAYERS = 6
C_IN = 256
GROWTH = 32
EPS = 1e-5
N_CORES = 8
IMGS_PER_CORE = 4
H = W = 56
HP = H + 2  # 58
SP = HP * HP  # 3364 padded spatial
GUARD = 64
SPG = GUARD + SP + GUARD
ROWS_PER_TILE = 8
TILE_N = ROWS_PER_TILE * HP  # 464, row-aligned: tile t covers grid rows 1+8t..9+8t
N_TILES = H // ROWS_PER_TILE  # 7 (covers all 56 interior rows)
FP16 = mybir.dt.float16
F32 = mybir.dt.float32

LAYER_C = [C_IN + i * GROWTH for i in range(NUM_LAYERS)]  # 256..416
LAYER_CHUNKS = [
    [128] * (c // 128) + ([c % 128] if c % 128 else []) for c in LAYER_C
]
P0 = [(C_IN + i * GROWTH) % 128 for i in range(NUM_LAYERS)]  # 0,32,64,96,0,32
CI = [(C_IN + i * GROWTH) // 128 for i in range(NUM_LAYERS)]  # 2,2,2,2,3,3
OFFS = [HP * (dy - 1) + (dx - 1) for dy in range(3) for dx in range(3)]
# per-layer (offset, chunk) instance lists and global weight-column indexes
LAYER_INSTS = []
_idx = 0
LAYER_IBASE = []
for _i in range(NUM_LAYERS):
    LAYER_IBASE.append(_idx)
    insts = [(o, c) for o in range(9) for c in range(len(LAYER_CHUNKS[_i]))]
    LAYER_INSTS.append(insts)
    _idx += len(insts)
N_INST = _idx  # 162

_COMPILED = None


def _install_ntff_hook():
    """Register the axon NTFF profile hook if the image's antenv lacks it."""
    try:
        import antenv.axon_hooks  # noqa: F401

        return
    except ImportError:
        pass
    try:
        import antenv
        from trn_agent_boot.trn_boot import _ntff_profile_via_ctypes

        hooks = types.ModuleType("antenv.axon_hooks")
        store = {}
        hooks.set_axon_ntff_profile_hook = lambda h: store.__setitem__("h", h)
        hooks.get_axon_ntff_profile_hook = lambda: store.get("h")
        antenv.axon_hooks = hooks
        sys.modules["antenv.axon_hooks"] = hooks
        hooks.set_axon_ntff_profile_hook(
            _ntff_profile_via_ctypes("/opt/axon/libaxon_pjrt.so")
        )
    except Exception:
        pass


def _interior(ap):
    """[P, SP]-flat AP -> [P, 56, 56] interior view of the padded 58x58 grid."""
    return ap.rearrange("p (r q) -> p r q", q=HP)[:, 1 : 1 + H, 1 : 1 + W]


def _build():
    nc = bacc.Bacc(None, target_bir_lowering=False, debug=False)

    x_d = nc.declare_dram_parameter(
        "x", [IMGS_PER_CORE, 2, 128, SP], F32, isOutput=False
    )
    w_d = nc.declare_dram_parameter("w", [128, N_INST, 32], FP16, isOutput=False)
    sel_d = nc.declare_dram_parameter("sel", [128, 4, 128], FP16, isOutput=False)
    bn_d = nc.declare_dram_parameter(
        "bn", [128, NUM_LAYERS, 4, 2], F32, isOutput=False
    )
    out_d = nc.declare_dram_parameter(
        "out", [IMGS_PER_CORE, 192, SP], F32, isOutput=True
    )

    with tile.TileContext(nc) as tc:
        with (
            tc.tile_pool(name="persist", bufs=1) as persist,
            tc.tile_pool(name="stage", bufs=8) as stage_pool,
            tc.tile_pool(name="acc", bufs=4, space="PSUM") as acc_pool,
            tc.tile_pool(name="red", bufs=4, space="PSUM") as red_pool,
        ):
            w_sb = persist.tile([128, N_INST, 32], FP16)
            sel_sb = persist.tile([128, 4, 128], FP16)
            bn_sb = persist.tile([128, NUM_LAYERS, 4, 2], F32)
            # x chunks double-buffered by image parity; grown chunks 2,3
            xbuf = persist.tile([128, 2, 2, SP], F32)
            grow = persist.tile([128, 2, SP], F32)
            hbuf = [persist.tile([128, 4, SPG], FP16, name=f"h{p}") for p in (0, 1)]

            def feat_chunk(img, c):
                return xbuf[:, img % 2, c, :] if c < 2 else grow[:, c - 2, :]

            nc.sync.dma_start(out=w_sb[:], in_=w_d[:])
            nc.sync.dma_start(out=sel_sb[:], in_=sel_d[:])
            nc.sync.dma_start(out=bn_sb[:], in_=bn_d[:])
            # full zero-init: K-padded matmuls read every row, and
            # uninitialized SBUF can hold NaN patterns (NaN * 0 = NaN)
            for hb in hbuf:
                nc.gpsimd.memset(hb[:], 0.0)
            # grow border rows are DMA'd out (host slices them off) but
            # never computed; zero once so reads are initialized
            nc.vector.memset(grow[:, :, 0:HP], 0.0)
            nc.vector.memset(grow[:, :, HP * (HP - 1) :], 0.0)

            def tile_range(t):
                """Flat offset (into SP) and length of row-aligned tile t."""
                return HP * (1 + ROWS_PER_TILE * t), TILE_N

            def rowwise(ap_flat):
                """[P, TILE_N] flat window -> [P, 8, 56] interior-columns view."""
                return ap_flat.rearrange("p (r q) -> p r q", q=HP)[:, :, 1 : 1 + W]

            for img in range(IMGS_PER_CORE):
                # load x (interior of padded layout), double-buffered by parity
                for c in range(2):
                    nc.sync.dma_start(
                        out=feat_chunk(img, c), in_=x_d[img, c]
                    )

                def emit_tail(i, t):
                    """Post-round ops for (layer i, tile t): cast partials,
                    selection-reduce, copy raw to feat, fused BN+ReLU of the
                    fresh 32 channels into the NEXT layer's h buffer."""
                    p0, ci = P0[i], CI[i]
                    acc, stg, red = tile_state[(i, t)]
                    o0, nt = tile_range(t)
                    nc.vector.tensor_copy(out=stg[:, :nt], in_=acc[:, :nt])
                    nc.tensor.matmul(
                        out=red[p0 : p0 + 32, :nt],
                        lhsT=sel_sb[:, p0 // 32, p0 : p0 + 32],
                        rhs=stg[:, :nt],
                        start=True,
                        stop=True,
                        tile_position=(0, p0),
                    )
                    nc.vector.tensor_copy(
                        out=grow[p0 : p0 + 32, ci - 2, o0 : o0 + nt],
                        in_=red[p0 : p0 + 32, :nt],
                    )
                    if i + 1 < NUM_LAYERS:
                        hn = hbuf[(i + 1) % 2]
                        nc.scalar.activation(
                            out=rowwise(
                                hn[p0 : p0 + 32, ci, GUARD + o0 : GUARD + o0 + nt]
                            ),
                            in_=rowwise(grow[p0 : p0 + 32, ci - 2, o0 : o0 + nt]),
                            func=mybir.ActivationFunctionType.Relu,
                            scale=bn_sb[p0 : p0 + 32, i + 1, ci, 0:1],
                            bias=bn_sb[p0 : p0 + 32, i + 1, ci, 1:2],
                        )

                tile_state = {}
                pending = []  # (layer, tile) whose tails are not yet emitted
                for i in range(NUM_LAYERS):
                    chunks = LAYER_CHUNKS[i]
                    h = hbuf[i % 2]
                    # full-chunk BN+ReLU for all but the freshest 32 channels
                    # (those were produced fused from PSUM by the prior layer);
                    # split into two row-halves for finer-grained pipelining
                    for c, kc in enumerate(chunks):
                        if i >= 1 and c == CI[i - 1]:
                            kc = P0[i - 1]  # exclude prior layer's fresh rows
                        if kc == 0:
                            continue
                        fc = feat_chunk(img, c)
                        for hi, (r0, r1) in enumerate(((0, H // 2), (H // 2, H))):
                            src_v = fc.rearrange("p (r q) -> p r q", q=HP)[
                                :kc, 1 + r0 : 1 + r1, 1 : 1 + W
                            ]
                            dst_v = h[:kc, c, GUARD : GUARD + SP].rearrange(
                                "p (r q) -> p r q", q=HP
                            )[:, 1 + r0 : 1 + r1, 1 : 1 + W]
                            if c == 1 and hi == 1:
                                # DVE path: fp32 tensor_scalar runs 2x_2P,
                                # fp16 in-place max runs 4x
                                nc.vector.tensor_scalar(
                                    out=dst_v,
                                    in0=src_v,
                                    scalar1=bn_sb[:kc, i, c, 0:1],
                                    scalar2=bn_sb[:kc, i, c, 1:2],
                                    op0=mybir.AluOpType.mult,
                                    op1=mybir.AluOpType.add,
                                )
                                nc.vector.tensor_scalar_max(
                                    out=dst_v, in0=dst_v, scalar1=0.0
                                )
                            else:
                                nc.scalar.activation(
                                    out=dst_v,
                                    in_=src_v,
                                    func=mybir.ActivationFunctionType.Relu,
                                    scale=bn_sb[:kc, i, c, 0:1],
                                    bias=bn_sb[:kc, i, c, 1:2],
                                )

                    insts = LAYER_INSTS[i]
                    ibase = LAYER_IBASE[i]
                    ngrp = [len(insts[g::4]) for g in range(4)]
                    for t in range(N_TILES):
                        o0, nt = tile_range(t)
                        hb0 = GUARD + o0
                        acc = acc_pool.tile([128, TILE_N], F32)
                        stg = stage_pool.tile([128, TILE_N], FP16)
                        red = red_pool.tile([128, TILE_N], F32)
                        tile_state[(i, t)] = (acc, stg, red)
                        # round-major emission: groups run concurrently.
                        # All K padded to 128 (zero weights / zero h rows):
                        # sub-128-row tiles serialize the whole round on HW.
                        for r in range(max(ngrp)):
                            for g in range(4):
                                j = r * 4 + g
                                if j >= len(insts):
                                    continue
                                o, c = insts[j]
                                nc.tensor.matmul(
                                    out=acc[32 * g : 32 * g + 32, :nt],
                                    lhsT=w_sb[:, ibase + j, :],
                                    rhs=h[:, c, hb0 + OFFS[o] : hb0 + OFFS[o] + nt],
                                    start=(r == 0),
                                    stop=(r == ngrp[g] - 1),
                                    tile_position=(0, 32 * g),
                                    skip_group_check=True,
                                )
                        # tails trail the rounds by two tiles so their
                        # PSUM->SBUF casts never stall the PE
                        pending.append((i, t))
                        if len(pending) > 3:
                            emit_tail(*pending.pop(0))
                    # flush to depth 1 at layer end: puts this layer's last
                    # fused relus ahead of the next layer's chunk relus in
                    # the ScalarE queue while keeping one tail in flight
                    while len(pending) > 1:
                        emit_tail(*pending.pop(0))
                for p in pending:
                    emit_tail(*p)

                # write out the 192 new channels (padded; host slices)
                nc.sync.dma_start(out=out_d[img, 0:128], in_=grow[:, 0, :])
                nc.sync.dma_start(out=out_d[img, 128:192], in_=grow[:64, 1, :])

    nc.compile()
    return nc


def _get_compiled():
    global _COMPILED
    if _COMPILED is None:
        _COMPILED = _build()
    return _COMPILED


def _prep_shared(bn_weights, bn_biases, bn_means, bn_vars, conv_kernels):
    """Weights / selection / BN tensors (identical on every core)."""
    w_all = np.zeros((128, N_INST, 32), dtype=np.float16)
    for i in range(NUM_LAYERS):
        wk = np.asarray(conv_kernels[i], dtype=np.float32)  # [32, C, 3, 3]
        for j, (o, c) in enumerate(LAYER_INSTS[i]):
            dy, dx = o // 3, o % 3
            kc = LAYER_CHUNKS[i][c]
            blk = wk[:, 128 * c : 128 * c + kc, dy, dx]  # [32, kc]
            w_all[:kc, LAYER_IBASE[i] + j, :] = blk.T.astype(np.float16)

    sel = np.zeros((128, 4, 128), dtype=np.float16)
    for v in range(4):
        for g in range(4):
            for m in range(32):
                sel[32 * g + m, v, 32 * v + m] = 1.0

    bn = np.zeros((128, NUM_LAYERS, 4, 2), dtype=np.float32)
    for i in range(NUM_LAYERS):
        s = np.asarray(bn_weights[i], np.float32) / np.sqrt(
            np.asarray(bn_vars[i], np.float32) + EPS
        )
        t = np.asarray(bn_biases[i], np.float32) - np.asarray(
            bn_means[i], np.float32
        ) * s
        for c, kc in enumerate(LAYER_CHUNKS[i]):
            bn[:kc, i, c, 0] = s[128 * c : 128 * c + kc]
            bn[:kc, i, c, 1] = t[128 * c : 128 * c + kc]
    return w_all, sel, bn


def _run(inputs, trace=False):
    _install_ntff_hook()
    nc = _get_compiled()
    x = np.asarray(inputs["x"], dtype=np.float32)
    n = x.shape[0]
    w_all, sel, bn = _prep_shared(
        inputs["bn_weights"],
        inputs["bn_biases"],
        inputs["bn_means"],
        inputs["bn_vars"],
        inputs["conv_kernels"],
    )
    xr = x.reshape(n, 2, 128, H, W)
    xpad = np.zeros((n, 2, 128, HP, HP), dtype=np.float32)
    xpad[:, :, :, 1 : 1 + H, 1 : 1 + W] = xr
    xpad = xpad.reshape(n, 2, 128, SP)
    in_maps = []
    for core in range(N_CORES):
        sl = xpad[core * IMGS_PER_CORE : (core + 1) * IMGS_PER_CORE]
        in_maps.append(
            {"x": np.ascontiguousarray(sl), "w": w_all, "sel": sel, "bn": bn}
        )
    res = run_bass_kernel_spmd(nc, in_maps, list(range(N_CORES)), trace=trace)
    out = np.empty((n, C_IN + NUM_LAYERS * GROWTH, H, W), dtype=np.float32)
    out[:, :C_IN] = x.reshape(n, C_IN, H, W)
    for core in range(N_CORES):
        new = res.results[core]["out"].reshape(IMGS_PER_CORE, 192, HP, HP)
        out[core * IMGS_PER_CORE : (core + 1) * IMGS_PER_CORE, C_IN:] = new[
            :, :, 1 : 1 + H, 1 : 1 + W
        ]
    return out, res


def kernel(**inputs) -> np.ndarray:
    out, _ = _run(inputs, trace=False)
    return out


# revision 21
# speedup vs baseline: 1.0440x; 1.0022x over previous
"""DenseNet block (6 layers, growth 32) on 8 TRN2 NeuronCores.

Data-parallel over batch: 32 images -> 4 per core. Per core, per image:
  features live in SBUF as f32, channels on partitions, spatial flattened
  in a zero-padded 58x58 layout; channel count grows 256 -> 448.
  Per layer:
    h = relu(feat * scale + shift)  (BN folded host-side) -> fp16, computed
      on ScalarE (activation) with a slice on VectorE (tensor_scalar 2x_2P),
      into parity-alternating zero-bordered h buffers. The freshest 32
      channels are instead produced per-tile right after their conv.
    conv 3x3 = 9 shifted matmuls per 128-channel K-chunk over row-aligned
      spatial tiles (8 rows x 58 = 464 cols = one PSUM bank), fp16 operands,
      f32 PSUM accumulation. M=32 output channels -> 4-way col-group packing
      (tile_position=(0,32g)) over (offset, chunk) instances; K always padded
      to 128 (zero weights) because sub-128-row PE tiles serialize a round.
      A selection-matrix matmul (M=32) reduces the 4 partials onto the
      partition range where the new channels live; VectorE evacuates PSUM.
  Tails (cast/reduce/copy/fused-relu) trail the matmul rounds by 3 tiles so
  the PE never waits on PSUM evacuation.
  Output: the 192 new channels DMA out padded; the host slices the interior
  and prepends x unchanged.
"""

import sys
import types

import numpy as np

import concourse.bass as bass
import concourse.tile as tile
from concourse import bacc, mybir
from concourse.bass_utils import run_bass_kernel_spmd

# ---------------------------------------------------------------- constants
NUM_LAYERS = 6
C_IN = 256
GROWTH = 32
EPS = 1e-5
N_CORES = 8
IMGS_PER_CORE = 4
H = W = 56
HP = H + 2  # 58
SP = HP * HP  # 3364 padded spatial
GUARD = 64
SPG = GUARD + SP + GUARD
ROWS_PER_TILE = 8
TILE_N = ROWS_PER_TILE * HP  # 464, row-aligned: tile t covers grid rows 1+8t..9+8t
N_TILES = H // ROWS_PER_TILE  # 7 (covers all 56 interior rows)
FP16 = mybir.dt.float16
F32 = mybir.dt.float32

LAYER_C = [C_IN + i * GROWTH for i in range(NUM_LAYERS)]  # 256..416
LAYER_CHUNKS = [
    [128] * (c // 128) + ([c % 128] if c % 128 else []) for c in LAYER_C
]
P0 = [(C_IN + i * GROWTH) % 128 for i in range(NUM_LAYERS)]  # 0,32,64,96,0,32
CI = [(C_IN + i * GROWTH) // 128 for i in range(NUM_LAYERS)]  # 2,2,2,2,3,3
OFFS = [HP * (dy - 1) + (dx - 1) for dy in range(3) for dx in range(3)]
# per-layer (offset, chunk) instance lists and global weight-column indexes
LAYER_INSTS = []
_idx = 0
LAYER_IBASE = []
for _i in range(NUM_LAYERS):
    LAYER_IBASE.append(_idx)
    insts = [(o, c) for o in range(9) for c in range(len(LAYER_CHUNKS[_i]))]
    LAYER_INSTS.append(insts)
    _idx += len(insts)
N_INST = _idx  # 162

_COMPILED = None


def _install_ntff_hook():
    """Register the axon NTFF profile hook if the image's antenv lacks it."""
    try:
        import antenv.axon_hooks  # noqa: F401

        return
    except ImportError:
        pass
    try:
        import antenv
        from trn_agent_boot.trn_boot import _ntff_profile_via_ctypes

        hooks = types.ModuleType("antenv.axon_hooks")
        store = {}
        hooks.set_axon_ntff_profile_hook = lambda h: store.__setitem__("h", h)
        hooks.get_axon_ntff_profile_hook = lambda: store.get("h")
        antenv.axon_hooks = hooks
        sys.modules["antenv.axon_hooks"] = hooks
        hooks.set_axon_ntff_profile_hook(
            _ntff_profile_via_ctypes("/opt/axon/libaxon_pjrt.so")
        )
    except Exception:
        pass


def _interior(ap):
    """[P, SP]-flat AP -> [P, 56, 56] interior view of the padded 58x58 grid."""
    return ap.rearrange("p (r q) -> p r q", q=HP)[:, 1 : 1 + H, 1 : 1 + W]


def _build():
    nc = bacc.Bacc(None, target_bir_lowering=False, debug=False)

    x_d = nc.declare_dram_parameter(
        "x", [IMGS_PER_CORE, 2, 128, SP], F32, isOutput=False
    )
    w_d = nc.declare_dram_parameter("w", [128, N_INST, 32], FP16, isOutput=False)
    sel_d = nc.declare_dram_parameter("sel", [128, 4, 128], FP16, isOutput=False)
    bn_d = nc.declare_dram_parameter(
        "bn", [128, NUM_LAYERS, 4, 2], F32, isOutput=False
    )
    out_d = nc.declare_dram_parameter(
        "out", [IMGS_PER_CORE, 192, SP], F32, isOutput=True
    )

    with tile.TileContext(nc) as tc:
        with (
            tc.tile_pool(name="persist", bufs=1) as persist,
            tc.tile_pool(name="stage", bufs=8) as stage_pool,
            tc.tile_pool(name="acc", bufs=4, space="PSUM") as acc_pool,
            tc.tile_pool(name="red", bufs=4, space="PSUM") as red_pool,
        ):
            w_sb = persist.tile([128, N_INST, 32], FP16)
            sel_sb = persist.tile([128, 4, 128], FP16)
            bn_sb = persist.tile([128, NUM_LAYERS, 4, 2], F32)
            # x chunks double-buffered by image parity; grown chunks 2,3
            xbuf = persist.tile([128, 2, 2, SP], F32)
            grow = persist.tile([128, 2, SP], F32)
            hbuf = [persist.tile([128, 4, SPG], FP16, name=f"h{p}") for p in (0, 1)]

            def feat_chunk(img, c):
                return xbuf[:, img % 2, c, :] if c < 2 else grow[:, c - 2, :]

            nc.sync.dma_start(out=w_sb[:], in_=w_d[:])
            nc.sync.dma_start(out=sel_sb[:], in_=sel_d[:])
            nc.sync.dma_start(out=bn_sb[:], in_=bn_d[:])
            # full zero-init: K-padded matmuls read every row, and
            # uninitialized SBUF can hold NaN patterns (NaN * 0 = NaN)
            for hb in hbuf:
                nc.gpsimd.memset(hb[:], 0.0)
            # grow border rows are DMA'd out (host slices them off) but
            # never computed; zero once so reads are initialized
            nc.vector.memset(grow[:, :, 0:HP], 0.0)
            nc.vector.memset(grow[:, :, HP * (HP - 1) :], 0.0)

            def tile_range(t):
                """Flat offset (into SP) and length of row-aligned tile t."""
                return HP * (1 + ROWS_PER_TILE * t), TILE_N

            def rowwise(ap_flat):
                """[P, TILE_N] flat window -> [P, 8, 56] interior-columns view."""
                return ap_flat.rearrange("p (r q) -> p r q", q=HP)[:, :, 1 : 1 + W]

            for img in range(IMGS_PER_CORE):
                # load x (interior of padded layout), double-buffered by parity
                for c in range(2):
                    nc.sync.dma_start(
                        out=feat_chunk(img, c), in_=x_d[img, c]
                    )

                def emit_tail(i, t):
                    """Post-round ops for (layer i, tile t): cast partials,
                    selection-reduce, copy raw to feat, fused BN+ReLU of the
                    fresh 32 channels into the NEXT layer's h buffer."""
                    p0, ci = P0[i], CI[i]
                    acc, stg, red = tile_state[(i, t)]
                    o0, nt = tile_range(t)
                    nc.vector.tensor_copy(out=stg[:, :nt], in_=acc[:, :nt])
                    nc.tensor.matmul(
                        out=red[p0 : p0 + 32, :nt],
                        lhsT=sel_sb[:, p0 // 32, p0 : p0 + 32],
                        rhs=stg[:, :nt],
                        start=True,
                        stop=True,
                        tile_position=(0, p0),
                    )
                    nc.scalar.activation(
                        out=grow[p0 : p0 + 32, ci - 2, o0 : o0 + nt],
                        in_=red[p0 : p0 + 32, :nt],
                        func=mybir.ActivationFunctionType.Copy,
                    )
                    if i + 1 < NUM_LAYERS:
                        hn = hbuf[(i + 1) % 2]
                        nc.scalar.activation(
                            out=rowwise(
                                hn[p0 : p0 + 32, ci, GUARD + o0 : GUARD + o0 + nt]
                            ),
                            in_=rowwise(grow[p0 : p0 + 32, ci - 2, o0 : o0 + nt]),
                            func=mybir.ActivationFunctionType.Relu,
                            scale=bn_sb[p0 : p0 + 32, i + 1, ci, 0:1],
                            bias=bn_sb[p0 : p0 + 32, i + 1, ci, 1:2],
                        )

                tile_state = {}
                pending = []  # (layer, tile) whose tails are not yet emitted
                for i in range(NUM_LAYERS):
                    chunks = LAYER_CHUNKS[i]
                    h = hbuf[i % 2]
                    # full-chunk BN+ReLU for all but the freshest 32 channels
                    # (those were produced fused from PSUM by the prior layer);
                    # split into two row-halves for finer-grained pipelining
                    def xrelu_views(li, c, hi):
                        r0, r1 = ((0, H // 2), (H // 2, H))[hi]
                        fc = feat_chunk(img, c)
                        hh = hbuf[li % 2]
                        src_v = fc.rearrange("p (r q) -> p r q", q=HP)[
                            :, 1 + r0 : 1 + r1, 1 : 1 + W
                        ]
                        dst_v = hh[:, c, GUARD : GUARD + SP].rearrange(
                            "p (r q) -> p r q", q=HP
                        )[:, 1 + r0 : 1 + r1, 1 : 1 + W]
                        return src_v, dst_v

                    def emit_dve_xrelu(li, c, hi):
                        # DVE path: fp32 tensor_scalar runs 2x_2P, fp16
                        # in-place max runs 4x
                        src_v, dst_v = xrelu_views(li, c, hi)
                        nc.vector.tensor_scalar(
                            out=dst_v,
                            in0=src_v,
                            scalar1=bn_sb[:, li, c, 0:1],
                            scalar2=bn_sb[:, li, c, 1:2],
                            op0=mybir.AluOpType.mult,
                            op1=mybir.AluOpType.add,
                        )
                        nc.vector.tensor_scalar_max(
                            out=dst_v, in0=dst_v, scalar1=0.0
                        )

                    if i == 0:
                        for c in range(2):
                            for hi in range(2):
                                emit_dve_xrelu(0, c, hi)
                    # grow chunks stay on ScalarE (minus prior fresh rows)
                    for c, kc in enumerate(chunks):
                        if c < 2:
                            continue
                        if i >= 1 and c == CI[i - 1]:
                            kc = P0[i - 1]  # exclude prior layer's fresh rows
                        if kc == 0:
                            continue
                        fc = feat_chunk(img, c)
                        for hi, (r0, r1) in enumerate(((0, H // 2), (H // 2, H))):
                            src_v = fc.rearrange("p (r q) -> p r q", q=HP)[
                                :kc, 1 + r0 : 1 + r1, 1 : 1 + W
                            ]
                            dst_v = h[:kc, c, GUARD : GUARD + SP].rearrange(
                                "p (r q) -> p r q", q=HP
                            )[:, 1 + r0 : 1 + r1, 1 : 1 + W]
                            nc.scalar.activation(
                                out=dst_v,
                                in_=src_v,
                                func=mybir.ActivationFunctionType.Relu,
                                scale=bn_sb[:kc, i, c, 0:1],
                                bias=bn_sb[:kc, i, c, 1:2],
                            )
                    # queue next layer's x-chunk relus, one per tile below
                    dve_pending = (
                        [(i + 1, c, hi) for c in range(2) for hi in range(2)]
                        if i + 1 < NUM_LAYERS
                        else []
                    )

                    insts = LAYER_INSTS[i]
                    ibase = LAYER_IBASE[i]
                    ngrp = [len(insts[g::4]) for g in range(4)]
                    for t in range(N_TILES):
                        o0, nt = tile_range(t)
                        hb0 = GUARD + o0
                        acc = acc_pool.tile([128, TILE_N], F32)
                        stg = stage_pool.tile([128, TILE_N], FP16)
                        red = red_pool.tile([128, TILE_N], F32)
                        tile_state[(i, t)] = (acc, stg, red)
                        # round-major emission: groups run concurrently.
                        # All K padded to 128 (zero weights / zero h rows):
                        # sub-128-row tiles serialize the whole round on HW.
                        for r in range(max(ngrp)):
                            for g in range(4):
                                j = r * 4 + g
                                if j >= len(insts):
                                    continue
                                o, c = insts[j]
                                nc.tensor.matmul(
                                    out=acc[32 * g : 32 * g + 32, :nt],
                                    lhsT=w_sb[:, ibase + j, :],
                                    rhs=h[:, c, hb0 + OFFS[o] : hb0 + OFFS[o] + nt],
                                    start=(r == 0),
                                    stop=(r == ngrp[g] - 1),
                                    tile_position=(0, 32 * g),
                                    skip_group_check=True,
                                )
                        # tails trail the rounds by two tiles so their
                        # PSUM->SBUF casts never stall the PE
                        pending.append((i, t))
                        if len(pending) > 3:
                            emit_tail(*pending.pop(0))
                        if dve_pending:
                            emit_dve_xrelu(*dve_pending.pop(0))
                    # flush to depth 1 at layer end: puts this layer's last
                    # fused relus ahead of the next layer's chunk relus in
                    # the ScalarE queue while keeping one tail in flight
                    while len(pending) > 1:
                        emit_tail(*pending.pop(0))
                for p in pending:
                    emit_tail(*p)

                # write out the 192 new channels (padded; host slices)
                nc.sync.dma_start(out=out_d[img, 0:128], in_=grow[:, 0, :])
                nc.sync.dma_start(out=out_d[img, 128:192], in_=grow[:64, 1, :])

    nc.compile()
    return nc


def _get_compiled():
    global _COMPILED
    if _COMPILED is None:
        _COMPILED = _build()
    return _COMPILED


def _prep_shared(bn_weights, bn_biases, bn_means, bn_vars, conv_kernels):
    """Weights / selection / BN tensors (identical on every core)."""
    w_all = np.zeros((128, N_INST, 32), dtype=np.float16)
    for i in range(NUM_LAYERS):
        wk = np.asarray(conv_kernels[i], dtype=np.float32)  # [32, C, 3, 3]
        for j, (o, c) in enumerate(LAYER_INSTS[i]):
            dy, dx = o // 3, o % 3
            kc = LAYER_CHUNKS[i][c]
            blk = wk[:, 128 * c : 128 * c + kc, dy, dx]  # [32, kc]
            w_all[:kc, LAYER_IBASE[i] + j, :] = blk.T.astype(np.float16)

    sel = np.zeros((128, 4, 128), dtype=np.float16)
    for v in range(4):
        for g in range(4):
            for m in range(32):
                sel[32 * g + m, v, 32 * v + m] = 1.0

    bn = np.zeros((128, NUM_LAYERS, 4, 2), dtype=np.float32)
    for i in range(NUM_LAYERS):
        s = np.asarray(bn_weights[i], np.float32) / np.sqrt(
            np.asarray(bn_vars[i], np.float32) + EPS
        )
        t = np.asarray(bn_biases[i], np.float32) - np.asarray(
            bn_means[i], np.float32
        ) * s
        for c, kc in enumerate(LAYER_CHUNKS[i]):
            bn[:kc, i, c, 0] = s[128 * c : 128 * c + kc]
            bn[:kc, i, c, 1] = t[128 * c : 128 * c + kc]
    return w_all, sel, bn


def _run(inputs, trace=False):
    _install_ntff_hook()
    nc = _get_compiled()
    x = np.asarray(inputs["x"], dtype=np.float32)
    n = x.shape[0]
    w_all, sel, bn = _prep_shared(
        inputs["bn_weights"],
        inputs["bn_biases"],
        inputs["bn_means"],
        inputs["bn_vars"],
        inputs["conv_kernels"],
    )
    xr = x.reshape(n, 2, 128, H, W)
    xpad = np.zeros((n, 2, 128, HP, HP), dtype=np.float32)
    xpad[:, :, :, 1 : 1 + H, 1 : 1 + W] = xr
    xpad = xpad.reshape(n, 2, 128, SP)
    in_maps = []
    for core in range(N_CORES):
        sl = xpad[core * IMGS_PER_CORE : (core + 1) * IMGS_PER_CORE]
        in_maps.append(
            {"x": np.ascontiguousarray(sl), "w": w_all, "sel": sel, "bn": bn}
        )
    res = run_bass_kernel_spmd(nc, in_maps, list(range(N_CORES)), trace=trace)
    out = np.empty((n, C_IN + NUM_LAYERS * GROWTH, H, W), dtype=np.float32)
    out[:, :C_IN] = x.reshape(n, C_IN, H, W)
    for core in range(N_CORES):
        new = res.results[core]["out"].reshape(IMGS_PER_CORE, 192, HP, HP)
        out[core * IMGS_PER_CORE : (core + 1) * IMGS_PER_CORE, C_IN:] = new[
            :, :, 1 : 1 + H, 1 : 1 + W
        ]
    return out, res


def kernel(**inputs) -> np.ndarray:
    out, _ = _run(inputs, trace=False)
    return out


# revision 22
# speedup vs baseline: 1.0782x; 1.0328x over previous
"""DenseNet block (6 layers, growth 32) on 8 TRN2 NeuronCores.

Data-parallel over batch: 32 images -> 4 per core. Per core, per image:
  features live in SBUF as f32, channels on partitions, spatial flattened
  in a zero-padded 58x58 layout; channel count grows 256 -> 448.
  Per layer:
    h = relu(feat * scale + shift)  (BN folded host-side) -> fp16, computed
      on ScalarE (activation) with a slice on VectorE (tensor_scalar 2x_2P),
      into parity-alternating zero-bordered h buffers. The freshest 32
      channels are instead produced per-tile right after their conv.
    conv 3x3 = 9 shifted matmuls per 128-channel K-chunk over row-aligned
      spatial tiles (8 rows x 58 = 464 cols = one PSUM bank), fp16 operands,
      f32 PSUM accumulation. M=32 output channels -> 4-way col-group packing
      (tile_position=(0,32g)) over (offset, chunk) instances; K always padded
      to 128 (zero weights) because sub-128-row PE tiles serialize a round.
      A selection-matrix matmul (M=32) reduces the 4 partials onto the
      partition range where the new channels live; VectorE evacuates PSUM.
  Tails (cast/reduce/copy/fused-relu) trail the matmul rounds by 3 tiles so
  the PE never waits on PSUM evacuation.
  Output: the 192 new channels DMA out padded; the host slices the interior
  and prepends x unchanged.
"""

import sys
import types

import numpy as np

import concourse.bass as bass
import concourse.tile as tile
from concourse import bacc, mybir
from concourse.bass_utils import run_bass_kernel_spmd

# ---------------------------------------------------------------- constants
NUM_LAYERS = 6
C_IN = 256
GROWTH = 32
EPS = 1e-5
N_CORES = 8
IMGS_PER_CORE = 4
H = W = 56
HP = H + 2  # 58
SP = HP * HP  # 3364 padded spatial
GUARD = 64
SPG = GUARD + SP + GUARD
ROWS_PER_TILE = 8
TILE_N = ROWS_PER_TILE * HP  # 464, row-aligned: tile t covers grid rows 1+8t..9+8t
N_TILES = H // ROWS_PER_TILE  # 7 (covers all 56 interior rows)
FP16 = mybir.dt.float16
F32 = mybir.dt.float32

LAYER_C = [C_IN + i * GROWTH for i in range(NUM_LAYERS)]  # 256..416
LAYER_CHUNKS = [
    [128] * (c // 128) + ([c % 128] if c % 128 else []) for c in LAYER_C
]
P0 = [(C_IN + i * GROWTH) % 128 for i in range(NUM_LAYERS)]  # 0,32,64,96,0,32
CI = [(C_IN + i * GROWTH) // 128 for i in range(NUM_LAYERS)]  # 2,2,2,2,3,3
OFFS = [HP * (dy - 1) + (dx - 1) for dy in range(3) for dx in range(3)]
# per-layer (offset, chunk) instance lists and global weight-column indexes
LAYER_INSTS = []
_idx = 0
LAYER_IBASE = []
for _i in range(NUM_LAYERS):
    LAYER_IBASE.append(_idx)
    insts = [(o, c) for o in range(9) for c in range(len(LAYER_CHUNKS[_i]))]
    LAYER_INSTS.append(insts)
    _idx += len(insts)
N_INST = _idx  # 162

_COMPILED = None


def _install_ntff_hook():
    """Register the axon NTFF profile hook if the image's antenv lacks it."""
    try:
        import antenv.axon_hooks  # noqa: F401

        return
    except ImportError:
        pass
    try:
        import antenv
        from trn_agent_boot.trn_boot import _ntff_profile_via_ctypes

        hooks = types.ModuleType("antenv.axon_hooks")
        store = {}
        hooks.set_axon_ntff_profile_hook = lambda h: store.__setitem__("h", h)
        hooks.get_axon_ntff_profile_hook = lambda: store.get("h")
        antenv.axon_hooks = hooks
        sys.modules["antenv.axon_hooks"] = hooks
        hooks.set_axon_ntff_profile_hook(
            _ntff_profile_via_ctypes("/opt/axon/libaxon_pjrt.so")
        )
    except Exception:
        pass


def _interior(ap):
    """[P, SP]-flat AP -> [P, 56, 56] interior view of the padded 58x58 grid."""
    return ap.rearrange("p (r q) -> p r q", q=HP)[:, 1 : 1 + H, 1 : 1 + W]


def _build():
    nc = bacc.Bacc(None, target_bir_lowering=False, debug=False)

    x_d = nc.declare_dram_parameter(
        "x", [IMGS_PER_CORE, 2, 128, SP], F32, isOutput=False
    )
    w_d = nc.declare_dram_parameter("w", [128, N_INST, 32], FP16, isOutput=False)
    sel_d = nc.declare_dram_parameter("sel", [128, 4, 128], FP16, isOutput=False)
    bn_d = nc.declare_dram_parameter(
        "bn", [128, NUM_LAYERS, 4, 2], F32, isOutput=False
    )
    out_d = nc.declare_dram_parameter(
        "out", [IMGS_PER_CORE, 192, SP], F32, isOutput=True
    )

    with tile.TileContext(nc) as tc:
        with (
            tc.tile_pool(name="persist", bufs=1) as persist,
            tc.tile_pool(name="stage", bufs=8) as stage_pool,
            tc.tile_pool(name="acc", bufs=4, space="PSUM") as acc_pool,
            tc.tile_pool(name="red", bufs=4, space="PSUM") as red_pool,
        ):
            w_sb = persist.tile([128, N_INST, 32], FP16)
            sel_sb = persist.tile([128, 4, 128], FP16)
            bn_sb = persist.tile([128, NUM_LAYERS, 4, 2], F32)
            # x chunks double-buffered by image parity; grown chunks 2,3
            xbuf = persist.tile([128, 2, 2, SP], F32)
            grow = persist.tile([128, 2, SP], F32)
            hbuf = [persist.tile([128, 4, SPG], FP16, name=f"h{p}") for p in (0, 1)]

            def feat_chunk(img, c):
                return xbuf[:, img % 2, c, :] if c < 2 else grow[:, c - 2, :]

            nc.sync.dma_start(out=w_sb[:], in_=w_d[:])
            nc.sync.dma_start(out=sel_sb[:], in_=sel_d[:])
            nc.sync.dma_start(out=bn_sb[:], in_=bn_d[:])
            # full zero-init: K-padded matmuls read every row, and
            # uninitialized SBUF can hold NaN patterns (NaN * 0 = NaN)
            # h0 on DVE (fast - gates the first layer's matmuls);
            # h1 on GpSimd in parallel (not needed until layer 1)
            nc.vector.memset(hbuf[0][:], 0.0)
            nc.gpsimd.memset(hbuf[1][:], 0.0)
            # grow border rows are DMA'd out (host slices them off) but
            # never computed; zero once so reads are initialized
            nc.vector.memset(grow[:, :, 0:HP], 0.0)
            nc.vector.memset(grow[:, :, HP * (HP - 1) :], 0.0)

            def tile_range(t):
                """Flat offset (into SP) and length of row-aligned tile t."""
                return HP * (1 + ROWS_PER_TILE * t), TILE_N

            def rowwise(ap_flat):
                """[P, TILE_N] flat window -> [P, 8, 56] interior-columns view."""
                return ap_flat.rearrange("p (r q) -> p r q", q=HP)[:, :, 1 : 1 + W]

            for img in range(IMGS_PER_CORE):
                # load x (interior of padded layout), double-buffered by parity
                for c in range(2):
                    nc.sync.dma_start(
                        out=feat_chunk(img, c), in_=x_d[img, c]
                    )

                def emit_tail(i, t):
                    """Post-round ops for (layer i, tile t): cast partials,
                    selection-reduce, copy raw to feat, fused BN+ReLU of the
                    fresh 32 channels into the NEXT layer's h buffer."""
                    p0, ci = P0[i], CI[i]
                    acc, stg, red = tile_state[(i, t)]
                    o0, nt = tile_range(t)
                    nc.vector.tensor_copy(out=stg[:, :nt], in_=acc[:, :nt])
                    nc.tensor.matmul(
                        out=red[p0 : p0 + 32, :nt],
                        lhsT=sel_sb[:, p0 // 32, p0 : p0 + 32],
                        rhs=stg[:, :nt],
                        start=True,
                        stop=True,
                        tile_position=(0, p0),
                    )
                    nc.scalar.activation(
                        out=grow[p0 : p0 + 32, ci - 2, o0 : o0 + nt],
                        in_=red[p0 : p0 + 32, :nt],
                        func=mybir.ActivationFunctionType.Copy,
                    )
                    if i + 1 < NUM_LAYERS:
                        hn = hbuf[(i + 1) % 2]
                        nc.scalar.activation(
                            out=rowwise(
                                hn[p0 : p0 + 32, ci, GUARD + o0 : GUARD + o0 + nt]
                            ),
                            in_=rowwise(grow[p0 : p0 + 32, ci - 2, o0 : o0 + nt]),
                            func=mybir.ActivationFunctionType.Relu,
                            scale=bn_sb[p0 : p0 + 32, i + 1, ci, 0:1],
                            bias=bn_sb[p0 : p0 + 32, i + 1, ci, 1:2],
                        )

                tile_state = {}
                pending = []  # (layer, tile) whose tails are not yet emitted
                for i in range(NUM_LAYERS):
                    chunks = LAYER_CHUNKS[i]
                    h = hbuf[i % 2]
                    # full-chunk BN+ReLU for all but the freshest 32 channels
                    # (those were produced fused from PSUM by the prior layer);
                    # split into two row-halves for finer-grained pipelining
                    def xrelu_views(li, c, hi):
                        r0, r1 = ((0, H // 2), (H // 2, H))[hi]
                        fc = feat_chunk(img, c)
                        hh = hbuf[li % 2]
                        src_v = fc.rearrange("p (r q) -> p r q", q=HP)[
                            :, 1 + r0 : 1 + r1, 1 : 1 + W
                        ]
                        dst_v = hh[:, c, GUARD : GUARD + SP].rearrange(
                            "p (r q) -> p r q", q=HP
                        )[:, 1 + r0 : 1 + r1, 1 : 1 + W]
                        return src_v, dst_v

                    def emit_dve_xrelu(li, c, hi):
                        # DVE path: fp32 tensor_scalar runs 2x_2P, fp16
                        # in-place max runs 4x
                        src_v, dst_v = xrelu_views(li, c, hi)
                        nc.vector.tensor_scalar(
                            out=dst_v,
                            in0=src_v,
                            scalar1=bn_sb[:, li, c, 0:1],
                            scalar2=bn_sb[:, li, c, 1:2],
                            op0=mybir.AluOpType.mult,
                            op1=mybir.AluOpType.add,
                        )
                        nc.vector.tensor_scalar_max(
                            out=dst_v, in0=dst_v, scalar1=0.0
                        )

                    if i == 0:
                        for c in range(2):
                            for hi in range(2):
                                emit_dve_xrelu(0, c, hi)
                    # grow chunks stay on ScalarE (minus prior fresh rows)
                    for c, kc in enumerate(chunks):
                        if c < 2:
                            continue
                        if i >= 1 and c == CI[i - 1]:
                            kc = P0[i - 1]  # exclude prior layer's fresh rows
                        if kc == 0:
                            continue
                        fc = feat_chunk(img, c)
                        for hi, (r0, r1) in enumerate(((0, H // 2), (H // 2, H))):
                            src_v = fc.rearrange("p (r q) -> p r q", q=HP)[
                                :kc, 1 + r0 : 1 + r1, 1 : 1 + W
                            ]
                            dst_v = h[:kc, c, GUARD : GUARD + SP].rearrange(
                                "p (r q) -> p r q", q=HP
                            )[:, 1 + r0 : 1 + r1, 1 : 1 + W]
                            nc.scalar.activation(
                                out=dst_v,
                                in_=src_v,
                                func=mybir.ActivationFunctionType.Relu,
                                scale=bn_sb[:kc, i, c, 0:1],
                                bias=bn_sb[:kc, i, c, 1:2],
                            )
                    # queue next layer's x-chunk relus, one per tile below
                    dve_pending = (
                        [(i + 1, c, hi) for c in range(2) for hi in range(2)]
                        if i + 1 < NUM_LAYERS
                        else []
                    )

                    insts = LAYER_INSTS[i]
                    ibase = LAYER_IBASE[i]
                    ngrp = [len(insts[g::4]) for g in range(4)]
                    for t in range(N_TILES):
                        o0, nt = tile_range(t)
                        hb0 = GUARD + o0
                        acc = acc_pool.tile([128, TILE_N], F32)
                        stg = stage_pool.tile([128, TILE_N], FP16)
                        red = red_pool.tile([128, TILE_N], F32)
                        tile_state[(i, t)] = (acc, stg, red)
                        # round-major emission: groups run concurrently.
                        # All K padded to 128 (zero weights / zero h rows):
                        # sub-128-row tiles serialize the whole round on HW.
                        for r in range(max(ngrp)):
                            for g in range(4):
                                j = r * 4 + g
                                if j >= len(insts):
                                    continue
                                o, c = insts[j]
                                nc.tensor.matmul(
                                    out=acc[32 * g : 32 * g + 32, :nt],
                                    lhsT=w_sb[:, ibase + j, :],
                                    rhs=h[:, c, hb0 + OFFS[o] : hb0 + OFFS[o] + nt],
                                    start=(r == 0),
                                    stop=(r == ngrp[g] - 1),
                                    tile_position=(0, 32 * g),
                                    skip_group_check=True,
                                )
                        # tails trail the rounds by two tiles so their
                        # PSUM->SBUF casts never stall the PE
                        pending.append((i, t))
                        if len(pending) > 3:
                            emit_tail(*pending.pop(0))
                        if dve_pending:
                            emit_dve_xrelu(*dve_pending.pop(0))
                    # flush to depth 1 at layer end: puts this layer's last
                    # fused relus ahead of the next layer's chunk relus in
                    # the ScalarE queue while keeping one tail in flight
                    while len(pending) > 1:
                        emit_tail(*pending.pop(0))
                for p in pending:
                    emit_tail(*p)

                # write out the 192 new channels (padded; host slices)
                nc.sync.dma_start(out=out_d[img, 0:128], in_=grow[:, 0, :])
                nc.sync.dma_start(out=out_d[img, 128:192], in_=grow[:64, 1, :])

    nc.compile()
    return nc


def _get_compiled():
    global _COMPILED
    if _COMPILED is None:
        _COMPILED = _build()
    return _COMPILED


def _prep_shared(bn_weights, bn_biases, bn_means, bn_vars, conv_kernels):
    """Weights / selection / BN tensors (identical on every core)."""
    w_all = np.zeros((128, N_INST, 32), dtype=np.float16)
    for i in range(NUM_LAYERS):
        wk = np.asarray(conv_kernels[i], dtype=np.float32)  # [32, C, 3, 3]
        for j, (o, c) in enumerate(LAYER_INSTS[i]):
            dy, dx = o // 3, o % 3
            kc = LAYER_CHUNKS[i][c]
            blk = wk[:, 128 * c : 128 * c + kc, dy, dx]  # [32, kc]
            w_all[:kc, LAYER_IBASE[i] + j, :] = blk.T.astype(np.float16)

    sel = np.zeros((128, 4, 128), dtype=np.float16)
    for v in range(4):
        for g in range(4):
            for m in range(32):
                sel[32 * g + m, v, 32 * v + m] = 1.0

    bn = np.zeros((128, NUM_LAYERS, 4, 2), dtype=np.float32)
    for i in range(NUM_LAYERS):
        s = np.asarray(bn_weights[i], np.float32) / np.sqrt(
            np.asarray(bn_vars[i], np.float32) + EPS
        )
        t = np.asarray(bn_biases[i], np.float32) - np.asarray(
            bn_means[i], np.float32
        ) * s
        for c, kc in enumerate(LAYER_CHUNKS[i]):
            bn[:kc, i, c, 0] = s[128 * c : 128 * c + kc]
            bn[:kc, i, c, 1] = t[128 * c : 128 * c + kc]
    return w_all, sel, bn


def _run(inputs, trace=False):
    _install_ntff_hook()
    nc = _get_compiled()
    x = np.asarray(inputs["x"], dtype=np.float32)
    n = x.shape[0]
    w_all, sel, bn = _prep_shared(
        inputs["bn_weights"],
        inputs["bn_biases"],
        inputs["bn_means"],
        inputs["bn_vars"],
        inputs["conv_kernels"],
    )
    xr = x.reshape(n, 2, 128, H, W)
    xpad = np.zeros((n, 2, 128, HP, HP), dtype=np.float32)
    xpad[:, :, :, 1 : 1 + H, 1 : 1 + W] = xr
    xpad = xpad.reshape(n, 2, 128, SP)
    in_maps = []
    for core in range(N_CORES):
        sl = xpad[core * IMGS_PER_CORE : (core + 1) * IMGS_PER_CORE]
        in_maps.append(
            {"x": np.ascontiguousarray(sl), "w": w_all, "sel": sel, "bn": bn}
        )
    res = run_bass_kernel_spmd(nc, in_maps, list(range(N_CORES)), trace=trace)
    out = np.empty((n, C_IN + NUM_LAYERS * GROWTH, H, W), dtype=np.float32)
    out[:, :C_IN] = x.reshape(n, C_IN, H, W)
    for core in range(N_CORES):
        new = res.results[core]["out"].reshape(IMGS_PER_CORE, 192, HP, HP)
        out[core * IMGS_PER_CORE : (core + 1) * IMGS_PER_CORE, C_IN:] = new[
            :, :, 1 : 1 + H, 1 : 1 + W
        ]
    return out, res


def kernel(**inputs) -> np.ndarray:
    out, _ = _run(inputs, trace=False)
    return out
